# revision 60
# baseline (speedup 1.0000x reference)
"""GQA (16 Q heads / 4 KV heads, causal) for Trainium2, 8 NeuronCores.

Sharding: core = b*4 + j  (b = batch 0..1, j = KV-head group 0..3).
Each core computes attention for its batch b and its 4 Q heads (KV head j)
over the full 2048-token sequence, projects through its Wo row-slice, and a
per-chunk ReduceScatter(add) over the 4 cores of each batch leaves each core
with 128-token slices of the final output (returned as bf16, converted on
host).

Pipeline layout (all engines near-saturated, PE is the roofline):
  - projections in bf16 (x, Wqkv bf16; psum f32): qT/kT/vT on chip.
    Heads 2m, 2m+1 live on partition halves [0:64], [64:128]; kT is
    duplicated to both halves (one partition-shifted copy per chunk).
  - attention per (chunk c, head-pair round r): for each 128-token key tile
    i: QK matmul (bf16) -> exp on Act engine (2 heads per instruction,
    [128, 2, W]) -> PV matmul (bf16) accumulating [65, 512] (ones column of
    vnat gives the softmax denominator for free).  Causal masking: the
    diagonal 128x128 block is handled by pre-writing -1e30 mask into PSUM
    (DVE) and accumulating scores onto it (start=False); strictly-above
    tiles are skipped; in-chunk column trimming via f0.
  - software pipelining: qk(i+1) is emitted before pv(i) so the PE never
    waits on the exp; Wo projection of chunk c is drip-fed into chunk c+1's
    instruction stream (one (t,e) tile per key-tile iteration).
  - normalization: pv evicted to SBUF immediately (frees the PSUM bank);
    reciprocal of denominator (DVE) -> ones-matmul broadcast (PE, into a qk
    pool slot) -> fused multiply-multiply STT (DVE) writing attnoutT
    directly (shifted output partitions for the odd head; inputs share base
    0).  All of this is queued and drip-fed into the next round so it never
    gates the attention pipeline; Pool only runs the ReduceScatters.
  - Wo in f32r, y evicted to bf16 (Pool), per-chunk ReduceScatter in bf16
    (halves collective time; the 15us fixed cost per collective dominates).
"""

import sys

sys.path.insert(0, "/opt/trn_rl_repo")

import numpy as np
import ml_dtypes

import concourse.bass as bass
import concourse.mybir as mybir
import concourse.tile as tile
from concourse import bacc
from concourse.bass_utils import run_bass_kernel_spmd

F32 = mybir.dt.float32
F32R = mybir.dt.float32r
BF16 = mybir.dt.bfloat16
EXP = mybir.ActivationFunctionType.Exp
MULT = mybir.AluOpType.mult

B, N, E = 2, 2048, 1024
D = 64          # head dim
KT = 8          # 1024 channels / 128
NCH = 4         # n chunks of 512
GROUPS = [[0, 1, 2, 3], [4, 5, 6, 7]]

_NC_CACHE = {}


def build_program():
    nc = bacc.Bacc("TRN2", target_bir_lowering=False, debug=False)
    nc.num_devices = 8

    xT_d = nc.dram_tensor("xT", [E, N], BF16, kind="ExternalInput")
    wqkv_d = nc.dram_tensor("wqkv", [E, 384], BF16, kind="ExternalInput")
    wo_d = nc.dram_tensor("wo", [256, E], F32R, kind="ExternalInput")
    maskT_d = nc.dram_tensor("maskT", [128, 128], BF16, kind="ExternalInput")
    identN_d = nc.dram_tensor("identN", [128, 128], BF16, kind="ExternalInput")
    ident_d = nc.dram_tensor("ident", [128, 64], BF16, kind="ExternalInput")
    y_rs_d = nc.dram_tensor("y_rs", [NCH, 128, E], BF16, kind="ExternalOutput")

    with tile.TileContext(nc) as tc:
        with (
            tc.tile_pool(name="const", bufs=1) as cpool,
            tc.tile_pool(name="qkv", bufs=1) as qpool,
            tc.tile_pool(name="attn", bufs=2) as apool,
            tc.tile_pool(name="pT", bufs=6) as ppool,
            tc.tile_pool(name="recip", bufs=3) as rpool,
            tc.tile_pool(name="pvc", bufs=4) as vpool,
            tc.tile_pool(name="ysb", bufs=6) as ypool,
            tc.tile_pool(name="dram", bufs=1, space="DRAM") as dpool,
        ):
            xT_sb = cpool.tile([128, KT, N], BF16)
            wqkv_sb = cpool.tile([128, KT, 384], BF16)
            wo_sb = cpool.tile([128, 2, E], F32R)
            maskT_sb = cpool.tile([128, 128], BF16)
            identN_sb = cpool.tile([128, 128], BF16)
            ident_sb = cpool.tile([128, 64], BF16)
            ones_sb = cpool.tile([1, 64], F32R)
            nc.vector.memset(ones_sb[:].bitcast(F32), 1.0)
            dummy_sb = cpool.tile([1, 8], F32)

            # Preload the Exp activation table while the PE does projections.
            nc.vector.memset(dummy_sb[:], 0.0)
            nc.scalar.activation(dummy_sb[:], dummy_sb[:], EXP)

            # SP queue: wqkv and x chunk-0 interleaved per k (the first proj
            # matmul only needs k-tile 0), then x chunk 1.  DVE queue: mask,
            # ident, x chunks 2-3, wo — SP's ~565ns/issue would otherwise
            # delay late-chunk loads past their use.
            def dma_x(q, ch, k):
                q.dma_start(
                    xT_sb[:, k, ch * 512 : (ch + 1) * 512],
                    xT_d[k * 128 : (k + 1) * 128, ch * 512 : (ch + 1) * 512],
                )

            nc.gpsimd.dma_start(maskT_sb[:], maskT_d[:])
            nc.gpsimd.dma_start(identN_sb[:], identN_d[:])
            nc.gpsimd.dma_start(ident_sb[:], ident_d[:])
            for k in range(KT):
                nc.scalar.dma_start(
                    wqkv_sb[:, k, :], wqkv_d[k * 128 : (k + 1) * 128, :]
                )
            for k in range(KT):
                nc.sync.dma_start(
                    xT_sb[:, k, 0:512], xT_d[k * 128 : (k + 1) * 128, 0:512]
                )
            for k in range(KT):
                nc.sync.dma_start(
                    xT_sb[:, k, 512:1024], xT_d[k * 128 : (k + 1) * 128, 512:1024]
                )
            for k in range(KT):
                nc.sync.dma_start(
                    xT_sb[:, k, 1024:2048], xT_d[k * 128 : (k + 1) * 128, 1024:2048]
                )
            for k in range(2):
                nc.sync.dma_start(wo_sb[:, k, :], wo_d[k * 128 : (k + 1) * 128, :])

            # qT: [64h + d, r, n] = head 2r+h;  kT2: k duplicated on both halves
            qT_sb = qpool.tile([128, 2, N], BF16)
            kT2_sb = qpool.tile([128, N], BF16)
            vT_sb = qpool.tile([128, N], BF16)     # only partitions 64:128 used
            vnat_sb = qpool.tile([128, 16, 66], BF16)  # [:, t, 0:64]=v, [:, t, 64]=1
            nc.vector.memset(vnat_sb[:, :, 64:65], 1.0)

            # ---- chunk-0 projection upfront; chunks 1-3 are drained into
            # the attention instruction stream (PE has slack under the
            # Act-paced regime, so the serial projection phase shrinks to
            # one chunk) ----
            with (
                tc.tile_pool(name="proj_ps", bufs=2, space="PSUM") as proj_ps,
                tc.tile_pool(name="tr_ps", bufs=2, space="PSUM") as tr_ps,
            ):
                for c in range(1):
                    sl = slice(c * 512, (c + 1) * 512)
                    ps = [
                        proj_ps.tile([128, 512], F32, name=f"ps{m}", tag=f"ps{m}")
                        for m in range(3)
                    ]
                    for k in range(KT):
                        for m in range(3):
                            nc.tensor.matmul(
                                ps[m][:],
                                wqkv_sb[:, k, m * 128 : (m + 1) * 128],
                                xT_sb[:, k, sl],
                                start=(k == 0),
                                stop=(k == KT - 1),
                            )
                    nc.vector.tensor_copy(vT_sb[64:128, sl], ps[2][64:128, :])
                    for t in range(4 * c, 4 * c + 4):
                        tp = tr_ps.tile([128, 64], BF16)
                        nc.tensor.transpose(
                            tp[:],
                            vT_sb[64:128, t * 128 : (t + 1) * 128],
                            ident_sb[64:128, :],
                        )
                        nc.vector.tensor_copy(vnat_sb[:, t, 0:64], tp[:])
                    nc.vector.tensor_copy(kT2_sb[0:64, sl], ps[2][0:64, :])
                    nc.scalar.copy(kT2_sb[64:128, sl], ps[2][0:64, :])
                    nc.vector.tensor_copy(qT_sb[:, 0, sl], ps[0][:])
                    nc.vector.tensor_copy(qT_sb[:, 1, sl], ps[1][:])

            y_parts = [
                dpool.tile([512, E], BF16, name=f"y_part{c}", tag=f"y_part{c}")
                for c in range(NCH)
            ]
            y_rss = dpool.tile([NCH, 128, E], BF16, name="y_rss", tag="y_rss")

            attnouts = {}
            pending_wo = []   # deque of (c, t, e) closures state
            wo_done = [0] * NCH

            # PSUM banks: qk 3x2 + pv 2x1 = 8.  Wo projection tiles borrow qk
            # pool slot halves; normalization and Wo work is queued and
            # drip-fed into the next round's instruction stream.
            with (
                tc.tile_pool(name="qk_ps", bufs=3, space="PSUM") as qk_ps,
                tc.tile_pool(name="pv_ps", bufs=2, space="PSUM") as pv_ps,
            ):
                pending = []
                pending_proj = []

                def drain_one():
                    if pending_proj:
                        pending_proj.pop(0)()
                    elif pending:
                        pending.pop(0)()

                def queue_proj(c):
                    # chunk-c projection as qk-pool-slot closures: three
                    # m-block accumulations plus a direct natural-layout V
                    # projection (out [128 tokens, 64] per t-tile, so no PE
                    # transpose / extra PSUM pool is needed).
                    sl = slice(c * 512, (c + 1) * 512)

                    def m_closure(m):
                        def go():
                            ps = qk_ps.tile([128, 2, 512], F32, name="qk", tag="qk")
                            acc = ps[:, 0, :]
                            for k in range(KT):
                                nc.tensor.matmul(
                                    acc,
                                    wqkv_sb[:, k, m * 128 : (m + 1) * 128],
                                    xT_sb[:, k, sl],
                                    start=(k == 0),
                                    stop=(k == KT - 1),
                                )
                            if m < 2:
                                nc.vector.tensor_copy(qT_sb[:, m, sl], acc)
                            else:
                                nc.vector.tensor_copy(kT2_sb[0:64, sl], acc[0:64, :])
                                nc.scalar.copy(kT2_sb[64:128, sl], acc[0:64, :])
                        return go

                    def v_closure():
                        ps = qk_ps.tile([128, 2, 512], F32, name="qk", tag="qk")
                        for tl in range(4):
                            t = 4 * c + tl
                            for k in range(KT):
                                nc.tensor.matmul(
                                    ps[:, 0, tl * 64 : (tl + 1) * 64],
                                    xT_sb[:, k, t * 128 : (t + 1) * 128],
                                    wqkv_sb[:, k, 320:384],
                                    start=(k == 0),
                                    stop=(k == KT - 1),
                                )
                        for tl in range(4):
                            nc.vector.tensor_copy(
                                vnat_sb[:, 4 * c + tl, 0:64],
                                ps[:, 0, tl * 64 : (tl + 1) * 64],
                            )

                    for m in range(3):
                        pending_proj.append(m_closure(m))
                    pending_proj.append(v_closure)

                def emit_wo(c, t, e, yp, evict_act=False):
                    at = attnouts[c]
                    for kb in range(2):
                        nc.tensor.matmul(
                            yp,
                            at[:, kb, t * 128 : (t + 1) * 128],
                            wo_sb[:, kb, e * 512 : (e + 1) * 512],
                            start=(kb == 0),
                            stop=(kb == 1),
                        )
                    y_sb = ypool.tile([128, 512], BF16, name="y_sb", tag="y_sb")
                    if evict_act:
                        nc.scalar.copy(y_sb[:], yp)
                    else:
                        nc.vector.tensor_copy(y_sb[:], yp)
                    nc.sync.dma_start(
                        y_parts[c][t * 128 : (t + 1) * 128, e * 512 : (e + 1) * 512],
                        y_sb[:],
                    )
                    wo_done[c] += 1
                    if wo_done[c] == 8:
                        nc.gpsimd.collective_compute(
                            "ReduceScatter",
                            mybir.AluOpType.add,
                            replica_groups=GROUPS,
                            ins=[y_parts[c].opt()],
                            outs=[y_rss[c, :, :].opt()],
                        )

                def queue_wo(c, tiles, alt_evict=False):
                    # one closure per pair of (t, e) tiles sharing a qk slot
                    def go(pair=tuple(tiles), ea=alt_evict):
                        yp = qk_ps.tile([128, 2, 512], F32, name="qk", tag="qk")
                        for s, (t, e) in enumerate(pair):
                            emit_wo(c, t, e, yp[:, s, :], evict_act=(ea and s == 0))
                    pending.append(go)

                def queue_norm(c, r, h, pvc, at):
                    # broadcast 1/denominator across partitions with a tiny
                    # ones-matmul (PE has slack; keeps Pool out of the chain)
                    def go():
                        recip = rpool.tile([1, 512], F32R, name="recip", tag="recip")
                        with nc.allow_low_precision(reason="recip feeds f32r bcast mm"):
                            nc.vector.reciprocal(recip[:], pvc[64:65, :])
                        bcq = qk_ps.tile([128, 2, 512], F32, name="qk", tag="qk")
                        nc.tensor.matmul(
                            bcq[0:64, 0, :],
                            ones_sb[:],
                            recip[:],
                            start=True,
                            stop=True,
                        )
                        nc.vector.scalar_tensor_tensor(
                            out=at[64 * h : 64 * h + 64, r, :],
                            in0=pvc[0:64, :],
                            scalar=1.0,
                            in1=bcq[0:64, 0, :],
                            op0=MULT,
                            op1=MULT,
                        )
                    pending.append(go)

                for c in range(NCH):
                    # chunk-c projection must be fully emitted before its
                    # attention; then queue the next chunk's projection
                    while pending_proj:
                        pending_proj.pop(0)()
                    if c + 1 < NCH:
                        queue_proj(c + 1)
                    nst = 4 * c + 4
                    at = apool.tile([128, 2, 512], F32R, name=f"attnoutT{c}", tag="at")
                    attnouts[c] = at
                    for r in range(2):
                        pv = [None, None]

                        def emit_pv(entry, pv=pv):
                            pi, pf0, ppT = entry
                            for h in range(2):
                                if pv[h] is None:
                                    pv[h] = pv_ps.tile(
                                        [65, 512], F32, name=f"pv{h}", tag="pv"
                                    )
                                nc.tensor.matmul(
                                    pv[h][:, pf0:512],
                                    vnat_sb[:, pi, 0:65],
                                    ppT[:, h, pf0:512],
                                    start=(pi == 0),
                                    stop=(pi == nst - 1),
                                )

                        pend = []
                        for i in range(nst):
                            f0 = max(0, 128 * (i - 4 * c))
                            diag = i >= 4 * c
                            qk = qk_ps.tile([128, 2, 512], F32, name="qk", tag="qk")
                            for h in range(2):
                                base = 64 * h
                                nc.tensor.matmul(
                                    qk[:, h, f0:512],
                                    kT2_sb[base : base + 64, i * 128 : (i + 1) * 128],
                                    qT_sb[base : base + 64, r,
                                          c * 512 + f0 : (c + 1) * 512],
                                    start=True,
                                    stop=True,
                                    skip_group_check=diag,
                                )
                                if diag:
                                    # causal mask applied on the PE: accumulate
                                    # maskT.T @ I = -1e30 upper triangle onto
                                    # the diagonal 128x128 score block — keeps
                                    # the exp dependency chain PE-only.
                                    nc.tensor.matmul(
                                        qk[:, h, f0 : f0 + 128],
                                        maskT_sb[:],
                                        identN_sb[:],
                                        start=False,
                                        stop=True,
                                        skip_group_check=True,
                                    )
                            if len(pend) == 2:
                                emit_pv(pend.pop(0))
                            pT = ppool.tile([128, 2, 512], BF16, name="pT", tag="pT")
                            nc.scalar.activation(
                                pT[:, :, f0:512], qk[:, :, f0:512], EXP
                            )
                            pend.append((i, f0, pT))
                            if i % 2 == 1:
                                drain_one()
                        while pend:
                            emit_pv(pend.pop(0))
                        for h in range(2):
                            # evict pv to SBUF immediately: frees the PSUM
                            # bank for the next round without waiting on the
                            # recip/bcast/normalize chain; the two heads run
                            # on Act/DVE in parallel (Act idles at boundaries)
                            pvc = vpool.tile([65, 512], F32, name="pvc", tag="pvc")
                            if h == 0:
                                nc.scalar.copy(pvc[:], pv[h][:])
                            else:
                                nc.vector.tensor_copy(pvc[:], pv[h][:])
                            queue_norm(c, r, h, pvc, at)
                    for t in range(4):
                        queue_wo(c, [(t, 0), (t, 1)], alt_evict=(c == NCH - 1))
                # tail: drain everything left (chunk 3 normalize + Wo)
                rest = pending[:]
                pending.clear()
                for go in rest:
                    go()
                # deferred output DMAs: chunks 0-2 together (deps force them
                # after chunk-2's stores), chunk 3 alone so the post-RS tail
                # only pays for one 256KB transfer.
                nc.sync.dma_start(y_rs_d[0:3, :, :], y_rss[0:3, :, :])
                nc.sync.dma_start(y_rs_d[3, :, :], y_rss[3, :, :])

    nc.finalize()
    return nc


def get_program():
    if "nc" not in _NC_CACHE:
        _NC_CACHE["nc"] = build_program()
    return _NC_CACHE["nc"]


def make_in_maps(x, Wq, Wk, Wv, Wo):
    bf16 = ml_dtypes.bfloat16
    tri = np.where(
        np.arange(128)[:, None] <= np.arange(128)[None, :], 0.0, -1e30
    ).astype(np.float32)
    maskT = np.ascontiguousarray(tri.T).astype(bf16)
    identN = np.eye(128, dtype=np.float32).astype(bf16)
    ident = np.tile(np.eye(64, dtype=np.float32), (2, 1)).astype(bf16)
    xT = [np.ascontiguousarray(x[b].T).astype(bf16) for b in range(B)]
    in_maps = []
    for core in range(8):
        b, j = core // 4, core % 4
        wqkv = np.ascontiguousarray(
            np.concatenate(
                [
                    Wq[:, j * 256 : (j + 1) * 256] * (1.0 / np.sqrt(D)),
                    Wk[:, j * 64 : (j + 1) * 64],
                    Wv[:, j * 64 : (j + 1) * 64],
                ],
                axis=1,
            )
        ).astype(bf16)
        wo = np.ascontiguousarray(Wo[j * 256 : (j + 1) * 256, :]).astype(np.float32)
        in_maps.append(
            {"xT": xT[b], "wqkv": wqkv, "wo": wo, "maskT": maskT,
             "identN": identN, "ident": ident}
        )
    return in_maps


def gather_output(results):
    y = np.empty((B, N, E), dtype=np.float32)
    for core in range(8):
        b, j = core // 4, core % 4
        piece = np.asarray(results[core]["y_rs"]).astype(np.float32)
        for c in range(NCH):
            r0 = 512 * c + 128 * j
            y[b, r0 : r0 + 128, :] = piece[c]
    return y


def kernel(x, Wq, Wk, Wv, Wo, _trace=False, **trace_kwargs):
    nc = get_program()
    in_maps = make_in_maps(
        np.asarray(x), np.asarray(Wq), np.asarray(Wk), np.asarray(Wv), np.asarray(Wo)
    )
    res = run_bass_kernel_spmd(nc, in_maps, list(range(8)), trace=_trace, **trace_kwargs)
    out = gather_output(res.results)
    if _trace:
        return out, res
    return out


# revision 63
# speedup vs baseline: 1.0172x; 1.0172x over previous
"""GQA (16 Q heads / 4 KV heads, causal) for Trainium2, 8 NeuronCores.

Sharding: core = b*4 + j  (b = batch 0..1, j = KV-head group 0..3).
Each core computes attention for its batch b and its 4 Q heads (KV head j)
over the full 2048-token sequence, projects through its Wo row-slice, and a
per-chunk ReduceScatter(add) over the 4 cores of each batch leaves each core
with 128-token slices of the final output (returned as bf16, converted on
host).

Pipeline layout (all engines near-saturated, PE is the roofline):
  - projections in bf16 (x, Wqkv bf16; psum f32): qT/kT/vT on chip.
    Heads 2m, 2m+1 live on partition halves [0:64], [64:128]; kT is
    duplicated to both halves (one partition-shifted copy per chunk).
  - attention per (chunk c, head-pair round r): for each 128-token key tile
    i: QK matmul (bf16) -> exp on Act engine (2 heads per instruction,
    [128, 2, W]) -> PV matmul (bf16) accumulating [65, 512] (ones column of
    vnat gives the softmax denominator for free).  Causal masking: the
    diagonal 128x128 block is handled by pre-writing -1e30 mask into PSUM
    (DVE) and accumulating scores onto it (start=False); strictly-above
    tiles are skipped; in-chunk column trimming via f0.
  - software pipelining: qk(i+1) is emitted before pv(i) so the PE never
    waits on the exp; Wo projection of chunk c is drip-fed into chunk c+1's
    instruction stream (one (t,e) tile per key-tile iteration).
  - normalization: pv evicted to SBUF immediately (frees the PSUM bank);
    reciprocal of denominator (DVE) -> ones-matmul broadcast (PE, into a qk
    pool slot) -> fused multiply-multiply STT (DVE) writing attnoutT
    directly (shifted output partitions for the odd head; inputs share base
    0).  All of this is queued and drip-fed into the next round so it never
    gates the attention pipeline; Pool only runs the ReduceScatters.
  - Wo in f32r, y evicted to bf16 (Pool), per-chunk ReduceScatter in bf16
    (halves collective time; the 15us fixed cost per collective dominates).
"""

import sys

sys.path.insert(0, "/opt/trn_rl_repo")

import numpy as np
import ml_dtypes

import concourse.bass as bass
import concourse.mybir as mybir
import concourse.tile as tile
from concourse import bacc
from concourse.bass_utils import run_bass_kernel_spmd

F32 = mybir.dt.float32
F32R = mybir.dt.float32r
BF16 = mybir.dt.bfloat16
EXP = mybir.ActivationFunctionType.Exp
MULT = mybir.AluOpType.mult

B, N, E = 2, 2048, 1024
D = 64          # head dim
KT = 8          # 1024 channels / 128
NCH = 4         # n chunks of 512
GROUPS = [[0, 1, 2, 3], [4, 5, 6, 7]]

_NC_CACHE = {}


def build_program():
    nc = bacc.Bacc("TRN2", target_bir_lowering=False, debug=False)
    nc.num_devices = 8

    xT_d = nc.dram_tensor("xT", [E, N], BF16, kind="ExternalInput")
    wqkv_d = nc.dram_tensor("wqkv", [E, 384], BF16, kind="ExternalInput")
    wo_d = nc.dram_tensor("wo", [256, E], F32R, kind="ExternalInput")
    maskT_d = nc.dram_tensor("maskT", [128, 128], BF16, kind="ExternalInput")
    identN_d = nc.dram_tensor("identN", [128, 128], BF16, kind="ExternalInput")
    ident_d = nc.dram_tensor("ident", [128, 64], BF16, kind="ExternalInput")
    y_rs_d = nc.dram_tensor("y_rs", [NCH, 128, E], BF16, kind="ExternalOutput")

    with tile.TileContext(nc) as tc:
        with (
            tc.tile_pool(name="const", bufs=1) as cpool,
            tc.tile_pool(name="qkv", bufs=1) as qpool,
            tc.tile_pool(name="attn", bufs=2) as apool,
            tc.tile_pool(name="pT", bufs=6) as ppool,
            tc.tile_pool(name="recip", bufs=3) as rpool,
            tc.tile_pool(name="pvc", bufs=4) as vpool,
            tc.tile_pool(name="ysb", bufs=6) as ypool,
            tc.tile_pool(name="dram", bufs=1, space="DRAM") as dpool,
        ):
            xT_sb = cpool.tile([128, KT, N], BF16)
            wqkv_sb = cpool.tile([128, KT, 384], BF16)
            wo_sb = cpool.tile([128, 2, E], F32R)
            maskT_sb = cpool.tile([128, 128], BF16)
            identN_sb = cpool.tile([128, 128], BF16)
            ident_sb = cpool.tile([128, 64], BF16)
            ones_sb = cpool.tile([1, 64], F32R)
            nc.vector.memset(ones_sb[:].bitcast(F32), 1.0)
            dummy_sb = cpool.tile([1, 8], F32)

            # Preload the Exp activation table while the PE does projections.
            nc.vector.memset(dummy_sb[:], 0.0)
            nc.scalar.activation(dummy_sb[:], dummy_sb[:], EXP)

            # SP queue: wqkv and x chunk-0 interleaved per k (the first proj
            # matmul only needs k-tile 0), then x chunk 1.  DVE queue: mask,
            # ident, x chunks 2-3, wo — SP's ~565ns/issue would otherwise
            # delay late-chunk loads past their use.
            def dma_x(q, ch, k):
                q.dma_start(
                    xT_sb[:, k, ch * 512 : (ch + 1) * 512],
                    xT_d[k * 128 : (k + 1) * 128, ch * 512 : (ch + 1) * 512],
                )

            nc.gpsimd.dma_start(maskT_sb[:], maskT_d[:])
            nc.gpsimd.dma_start(identN_sb[:], identN_d[:])
            nc.gpsimd.dma_start(ident_sb[:], ident_d[:])
            for k in range(KT):
                nc.scalar.dma_start(
                    wqkv_sb[:, k, :], wqkv_d[k * 128 : (k + 1) * 128, :]
                )
            for k in range(KT):
                nc.sync.dma_start(
                    xT_sb[:, k, 0:512], xT_d[k * 128 : (k + 1) * 128, 0:512]
                )
            for k in range(KT):
                nc.sync.dma_start(
                    xT_sb[:, k, 512:1024], xT_d[k * 128 : (k + 1) * 128, 512:1024]
                )
            for k in range(KT):
                nc.sync.dma_start(
                    xT_sb[:, k, 1024:2048], xT_d[k * 128 : (k + 1) * 128, 1024:2048]
                )
            for k in range(2):
                nc.sync.dma_start(wo_sb[:, k, :], wo_d[k * 128 : (k + 1) * 128, :])

            # qT: [64h + d, r, n] = head 2r+h;  kT2: k duplicated on both halves
            qT_sb = qpool.tile([128, 2, N], BF16)
            kT2_sb = qpool.tile([128, N], BF16)
            vT_sb = qpool.tile([128, N], BF16)     # only partitions 64:128 used
            vnat_sb = qpool.tile([128, 16, 66], BF16)  # [:, t, 0:64]=v, [:, t, 64]=1
            nc.vector.memset(vnat_sb[:, :, 64:65], 1.0)

            # ---- chunk-0 projection upfront; chunks 1-3 are drained into
            # the attention instruction stream (PE has slack under the
            # Act-paced regime, so the serial projection phase shrinks to
            # one chunk) ----
            with (
                tc.tile_pool(name="proj_ps", bufs=2, space="PSUM") as proj_ps,
                tc.tile_pool(name="tr_ps", bufs=2, space="PSUM") as tr_ps,
            ):
                for c in range(2):
                    sl = slice(c * 512, (c + 1) * 512)
                    ps = [
                        proj_ps.tile([128, 512], F32, name=f"ps{m}", tag=f"ps{m}")
                        for m in range(3)
                    ]
                    for k in range(KT):
                        for m in range(3):
                            nc.tensor.matmul(
                                ps[m][:],
                                wqkv_sb[:, k, m * 128 : (m + 1) * 128],
                                xT_sb[:, k, sl],
                                start=(k == 0),
                                stop=(k == KT - 1),
                            )
                    nc.vector.tensor_copy(vT_sb[64:128, sl], ps[2][64:128, :])
                    for t in range(4 * c, 4 * c + 4):
                        tp = tr_ps.tile([128, 64], BF16)
                        nc.tensor.transpose(
                            tp[:],
                            vT_sb[64:128, t * 128 : (t + 1) * 128],
                            ident_sb[64:128, :],
                        )
                        nc.vector.tensor_copy(vnat_sb[:, t, 0:64], tp[:])
                    nc.vector.tensor_copy(kT2_sb[0:64, sl], ps[2][0:64, :])
                    nc.scalar.copy(kT2_sb[64:128, sl], ps[2][0:64, :])
                    nc.vector.tensor_copy(qT_sb[:, 0, sl], ps[0][:])
                    nc.vector.tensor_copy(qT_sb[:, 1, sl], ps[1][:])

            y_parts = [
                dpool.tile([512, E], BF16, name=f"y_part{c}", tag=f"y_part{c}")
                for c in range(NCH)
            ]
            y_rss = dpool.tile([NCH, 128, E], BF16, name="y_rss", tag="y_rss")

            attnouts = {}
            pending_wo = []   # deque of (c, t, e) closures state
            wo_done = [0] * NCH

            # PSUM banks: qk 3x2 + pv 2x1 = 8.  Wo projection tiles borrow qk
            # pool slot halves; normalization and Wo work is queued and
            # drip-fed into the next round's instruction stream.
            with (
                tc.tile_pool(name="qk_ps", bufs=3, space="PSUM") as qk_ps,
                tc.tile_pool(name="pv_ps", bufs=2, space="PSUM") as pv_ps,
            ):
                pending = []
                pending_proj = []

                def drain_one():
                    if pending_proj:
                        pending_proj.pop(0)()
                    elif pending:
                        pending.pop(0)()

                def queue_proj(c):
                    # chunk-c projection as qk-pool-slot closures: three
                    # m-block accumulations plus a direct natural-layout V
                    # projection (out [128 tokens, 64] per t-tile, so no PE
                    # transpose / extra PSUM pool is needed).
                    sl = slice(c * 512, (c + 1) * 512)

                    def m_closure(m):
                        def go():
                            ps = qk_ps.tile([128, 2, 512], F32, name="qk", tag="qk")
                            acc = ps[:, 0, :]
                            for k in range(KT):
                                nc.tensor.matmul(
                                    acc,
                                    wqkv_sb[:, k, m * 128 : (m + 1) * 128],
                                    xT_sb[:, k, sl],
                                    start=(k == 0),
                                    stop=(k == KT - 1),
                                )
                            if m < 2:
                                nc.vector.tensor_copy(qT_sb[:, m, sl], acc)
                            else:
                                nc.vector.tensor_copy(kT2_sb[0:64, sl], acc[0:64, :])
                                nc.scalar.copy(kT2_sb[64:128, sl], acc[0:64, :])
                        return go

                    def v_closure():
                        ps = qk_ps.tile([128, 2, 512], F32, name="qk", tag="qk")
                        for tl in range(4):
                            t = 4 * c + tl
                            for k in range(KT):
                                nc.tensor.matmul(
                                    ps[:, 0, tl * 64 : (tl + 1) * 64],
                                    xT_sb[:, k, t * 128 : (t + 1) * 128],
                                    wqkv_sb[:, k, 320:384],
                                    start=(k == 0),
                                    stop=(k == KT - 1),
                                )
                        for tl in range(4):
                            nc.vector.tensor_copy(
                                vnat_sb[:, 4 * c + tl, 0:64],
                                ps[:, 0, tl * 64 : (tl + 1) * 64],
                            )

                    for m in range(3):
                        pending_proj.append(m_closure(m))
                    pending_proj.append(v_closure)

                def emit_wo(c, t, e, yp, evict_act=False):
                    at = attnouts[c]
                    for kb in range(2):
                        nc.tensor.matmul(
                            yp,
                            at[:, kb, t * 128 : (t + 1) * 128],
                            wo_sb[:, kb, e * 512 : (e + 1) * 512],
                            start=(kb == 0),
                            stop=(kb == 1),
                        )
                    y_sb = ypool.tile([128, 512], BF16, name="y_sb", tag="y_sb")
                    if evict_act:
                        nc.scalar.copy(y_sb[:], yp)
                    else:
                        nc.vector.tensor_copy(y_sb[:], yp)
                    nc.sync.dma_start(
                        y_parts[c][t * 128 : (t + 1) * 128, e * 512 : (e + 1) * 512],
                        y_sb[:],
                    )
                    wo_done[c] += 1
                    if wo_done[c] == 8:
                        nc.gpsimd.collective_compute(
                            "ReduceScatter",
                            mybir.AluOpType.add,
                            replica_groups=GROUPS,
                            ins=[y_parts[c].opt()],
                            outs=[y_rss[c, :, :].opt()],
                        )

                def queue_wo(c, tiles, alt_evict=False):
                    # one closure per pair of (t, e) tiles sharing a qk slot
                    def go(pair=tuple(tiles), ea=alt_evict):
                        yp = qk_ps.tile([128, 2, 512], F32, name="qk", tag="qk")
                        for s, (t, e) in enumerate(pair):
                            emit_wo(c, t, e, yp[:, s, :], evict_act=(ea and s == 0))
                    pending.append(go)

                def queue_norm(c, r, h, pvc, at):
                    # broadcast 1/denominator across partitions with a tiny
                    # ones-matmul (PE has slack; keeps Pool out of the chain)
                    def go():
                        recip = rpool.tile([1, 512], F32R, name="recip", tag="recip")
                        with nc.allow_low_precision(reason="recip feeds f32r bcast mm"):
                            nc.vector.reciprocal(recip[:], pvc[64:65, :])
                        bcq = qk_ps.tile([128, 2, 512], F32, name="qk", tag="qk")
                        nc.tensor.matmul(
                            bcq[0:64, 0, :],
                            ones_sb[:],
                            recip[:],
                            start=True,
                            stop=True,
                        )
                        nc.vector.scalar_tensor_tensor(
                            out=at[64 * h : 64 * h + 64, r, :],
                            in0=pvc[0:64, :],
                            scalar=1.0,
                            in1=bcq[0:64, 0, :],
                            op0=MULT,
                            op1=MULT,
                        )
                    pending.append(go)

                for c in range(NCH):
                    # chunk-c projection must be fully emitted before its
                    # attention; then queue the next chunk's projection
                    while pending_proj:
                        pending_proj.pop(0)()
                    if 2 <= c + 1 < NCH:
                        queue_proj(c + 1)
                    nst = 4 * c + 4
                    at = apool.tile([128, 2, 512], F32R, name=f"attnoutT{c}", tag="at")
                    attnouts[c] = at
                    for r in range(2):
                        pv = [None, None]

                        def emit_pv(entry, pv=pv):
                            pi, pf0, ppT = entry
                            for h in range(2):
                                if pv[h] is None:
                                    pv[h] = pv_ps.tile(
                                        [65, 512], F32, name=f"pv{h}", tag="pv"
                                    )
                                nc.tensor.matmul(
                                    pv[h][:, pf0:512],
                                    vnat_sb[:, pi, 0:65],
                                    ppT[:, h, pf0:512],
                                    start=(pi == 0),
                                    stop=(pi == nst - 1),
                                )

                        pend = []
                        for i in range(nst):
                            f0 = max(0, 128 * (i - 4 * c))
                            diag = i >= 4 * c
                            qk = qk_ps.tile([128, 2, 512], F32, name="qk", tag="qk")
                            for h in range(2):
                                base = 64 * h
                                nc.tensor.matmul(
                                    qk[:, h, f0:512],
                                    kT2_sb[base : base + 64, i * 128 : (i + 1) * 128],
                                    qT_sb[base : base + 64, r,
                                          c * 512 + f0 : (c + 1) * 512],
                                    start=True,
                                    stop=True,
                                    skip_group_check=diag,
                                )
                                if diag:
                                    # causal mask applied on the PE: accumulate
                                    # maskT.T @ I = -1e30 upper triangle onto
                                    # the diagonal 128x128 score block — keeps
                                    # the exp dependency chain PE-only.
                                    nc.tensor.matmul(
                                        qk[:, h, f0 : f0 + 128],
                                        maskT_sb[:],
                                        identN_sb[:],
                                        start=False,
                                        stop=True,
                                        skip_group_check=True,
                                    )
                            if len(pend) == 2:
                                emit_pv(pend.pop(0))
                            pT = ppool.tile([128, 2, 512], BF16, name="pT", tag="pT")
                            nc.scalar.activation(
                                pT[:, :, f0:512], qk[:, :, f0:512], EXP
                            )
                            pend.append((i, f0, pT))
                            if pending_proj or i % 2 == 1:
                                drain_one()
                        while pend:
                            emit_pv(pend.pop(0))
                        for h in range(2):
                            # evict pv to SBUF immediately: frees the PSUM
                            # bank for the next round without waiting on the
                            # recip/bcast/normalize chain; the two heads run
                            # on Act/DVE in parallel (Act idles at boundaries)
                            pvc = vpool.tile([65, 512], F32, name="pvc", tag="pvc")
                            if h == 0:
                                nc.scalar.copy(pvc[:], pv[h][:])
                            else:
                                nc.vector.tensor_copy(pvc[:], pv[h][:])
                            queue_norm(c, r, h, pvc, at)
                    for t in range(4):
                        queue_wo(c, [(t, 0), (t, 1)], alt_evict=(c == NCH - 1))
                # tail: drain everything left (chunk 3 normalize + Wo)
                rest = pending[:]
                pending.clear()
                for go in rest:
                    go()
                # deferred output DMAs: chunks 0-2 together (deps force them
                # after chunk-2's stores), chunk 3 alone so the post-RS tail
                # only pays for one 256KB transfer.
                nc.sync.dma_start(y_rs_d[0:3, :, :], y_rss[0:3, :, :])
                nc.sync.dma_start(y_rs_d[3, :, :], y_rss[3, :, :])

    nc.finalize()
    return nc


def get_program():
    if "nc" not in _NC_CACHE:
        _NC_CACHE["nc"] = build_program()
    return _NC_CACHE["nc"]


def make_in_maps(x, Wq, Wk, Wv, Wo):
    bf16 = ml_dtypes.bfloat16
    tri = np.where(
        np.arange(128)[:, None] <= np.arange(128)[None, :], 0.0, -1e30
    ).astype(np.float32)
    maskT = np.ascontiguousarray(tri.T).astype(bf16)
    identN = np.eye(128, dtype=np.float32).astype(bf16)
    ident = np.tile(np.eye(64, dtype=np.float32), (2, 1)).astype(bf16)
    xT = [np.ascontiguousarray(x[b].T).astype(bf16) for b in range(B)]
    in_maps = []
    for core in range(8):
        b, j = core // 4, core % 4
        wqkv = np.ascontiguousarray(
            np.concatenate(
                [
                    Wq[:, j * 256 : (j + 1) * 256] * (1.0 / np.sqrt(D)),
                    Wk[:, j * 64 : (j + 1) * 64],
                    Wv[:, j * 64 : (j + 1) * 64],
                ],
                axis=1,
            )
        ).astype(bf16)
        wo = np.ascontiguousarray(Wo[j * 256 : (j + 1) * 256, :]).astype(np.float32)
        in_maps.append(
            {"xT": xT[b], "wqkv": wqkv, "wo": wo, "maskT": maskT,
             "identN": identN, "ident": ident}
        )
    return in_maps


def gather_output(results):
    y = np.empty((B, N, E), dtype=np.float32)
    for core in range(8):
        b, j = core // 4, core % 4
        piece = np.asarray(results[core]["y_rs"]).astype(np.float32)
        for c in range(NCH):
            r0 = 512 * c + 128 * j
            y[b, r0 : r0 + 128, :] = piece[c]
    return y


def kernel(x, Wq, Wk, Wv, Wo, _trace=False, **trace_kwargs):
    nc = get_program()
    in_maps = make_in_maps(
        np.asarray(x), np.asarray(Wq), np.asarray(Wk), np.asarray(Wv), np.asarray(Wo)
    )
    res = run_bass_kernel_spmd(nc, in_maps, list(range(8)), trace=_trace, **trace_kwargs)
    out = gather_output(res.results)
    if _trace:
        return out, res
    return out


# revision 64
# speedup vs baseline: 1.0229x; 1.0056x over previous
"""GQA (16 Q heads / 4 KV heads, causal) for Trainium2, 8 NeuronCores.

Sharding: core = b*4 + j  (b = batch 0..1, j = KV-head group 0..3).
Each core computes attention for its batch b and its 4 Q heads (KV head j)
over the full 2048-token sequence, projects through its Wo row-slice, and a
per-chunk ReduceScatter(add) over the 4 cores of each batch leaves each core
with 128-token slices of the final output (returned as bf16, converted on
host).

Pipeline layout (all engines near-saturated, PE is the roofline):
  - projections in bf16 (x, Wqkv bf16; psum f32): qT/kT/vT on chip.
    Heads 2m, 2m+1 live on partition halves [0:64], [64:128]; kT is
    duplicated to both halves (one partition-shifted copy per chunk).
  - attention per (chunk c, head-pair round r): for each 128-token key tile
    i: QK matmul (bf16) -> exp on Act engine (2 heads per instruction,
    [128, 2, W]) -> PV matmul (bf16) accumulating [65, 512] (ones column of
    vnat gives the softmax denominator for free).  Causal masking: the
    diagonal 128x128 block is handled by pre-writing -1e30 mask into PSUM
    (DVE) and accumulating scores onto it (start=False); strictly-above
    tiles are skipped; in-chunk column trimming via f0.
  - software pipelining: qk(i+1) is emitted before pv(i) so the PE never
    waits on the exp; Wo projection of chunk c is drip-fed into chunk c+1's
    instruction stream (one (t,e) tile per key-tile iteration).
  - normalization: pv evicted to SBUF immediately (frees the PSUM bank);
    reciprocal of denominator (DVE) -> ones-matmul broadcast (PE, into a qk
    pool slot) -> fused multiply-multiply STT (DVE) writing attnoutT
    directly (shifted output partitions for the odd head; inputs share base
    0).  All of this is queued and drip-fed into the next round so it never
    gates the attention pipeline; Pool only runs the ReduceScatters.
  - Wo in f32r, y evicted to bf16 (Pool), per-chunk ReduceScatter in bf16
    (halves collective time; the 15us fixed cost per collective dominates).
"""

import sys

sys.path.insert(0, "/opt/trn_rl_repo")

import numpy as np
import ml_dtypes

import concourse.bass as bass
import concourse.mybir as mybir
import concourse.tile as tile
from concourse import bacc
from concourse.bass_utils import run_bass_kernel_spmd

F32 = mybir.dt.float32
F32R = mybir.dt.float32r
BF16 = mybir.dt.bfloat16
EXP = mybir.ActivationFunctionType.Exp
MULT = mybir.AluOpType.mult

B, N, E = 2, 2048, 1024
D = 64          # head dim
KT = 8          # 1024 channels / 128
NCH = 4         # n chunks of 512
GROUPS = [[0, 1, 2, 3], [4, 5, 6, 7]]

_NC_CACHE = {}


def build_program():
    nc = bacc.Bacc("TRN2", target_bir_lowering=False, debug=False)
    nc.num_devices = 8

    xT_d = nc.dram_tensor("xT", [E, N], BF16, kind="ExternalInput")
    wqkv_d = nc.dram_tensor("wqkv", [E, 384], BF16, kind="ExternalInput")
    wo_d = nc.dram_tensor("wo", [256, E], F32R, kind="ExternalInput")
    maskT_d = nc.dram_tensor("maskT", [128, 128], BF16, kind="ExternalInput")
    identN_d = nc.dram_tensor("identN", [128, 128], BF16, kind="ExternalInput")
    ident_d = nc.dram_tensor("ident", [128, 64], BF16, kind="ExternalInput")
    y_rs_d = nc.dram_tensor("y_rs", [NCH, 128, E], BF16, kind="ExternalOutput")

    with tile.TileContext(nc) as tc:
        with (
            tc.tile_pool(name="const", bufs=1) as cpool,
            tc.tile_pool(name="qkv", bufs=1) as qpool,
            tc.tile_pool(name="attn", bufs=2) as apool,
            tc.tile_pool(name="pT", bufs=6) as ppool,
            tc.tile_pool(name="recip", bufs=3) as rpool,
            tc.tile_pool(name="pvc", bufs=4) as vpool,
            tc.tile_pool(name="ysb", bufs=6) as ypool,
            tc.tile_pool(name="dram", bufs=1, space="DRAM") as dpool,
        ):
            xT_sb = cpool.tile([128, KT, N], BF16)
            wqkv_sb = cpool.tile([128, KT, 384], BF16)
            wo_sb = cpool.tile([128, 2, E], F32R)
            maskT_sb = cpool.tile([128, 128], BF16)
            identN_sb = cpool.tile([128, 128], BF16)
            ident_sb = cpool.tile([128, 64], BF16)
            ones_sb = cpool.tile([1, 64], F32R)
            nc.vector.memset(ones_sb[:].bitcast(F32), 1.0)
            dummy_sb = cpool.tile([1, 8], F32)

            # Preload the Exp activation table while the PE does projections.
            nc.vector.memset(dummy_sb[:], 0.0)
            nc.scalar.activation(dummy_sb[:], dummy_sb[:], EXP)

            # SP queue: wqkv and x chunk-0 interleaved per k (the first proj
            # matmul only needs k-tile 0), then x chunk 1.  DVE queue: mask,
            # ident, x chunks 2-3, wo — SP's ~565ns/issue would otherwise
            # delay late-chunk loads past their use.
            def dma_x(q, ch, k):
                q.dma_start(
                    xT_sb[:, k, ch * 512 : (ch + 1) * 512],
                    xT_d[k * 128 : (k + 1) * 128, ch * 512 : (ch + 1) * 512],
                )

            nc.gpsimd.dma_start(maskT_sb[:], maskT_d[:])
            nc.gpsimd.dma_start(identN_sb[:], identN_d[:])
            nc.gpsimd.dma_start(ident_sb[:], ident_d[:])
            for k in range(KT):
                nc.scalar.dma_start(
                    wqkv_sb[:, k, :], wqkv_d[k * 128 : (k + 1) * 128, :]
                )
            for k in range(KT):
                nc.sync.dma_start(
                    xT_sb[:, k, 0:512], xT_d[k * 128 : (k + 1) * 128, 0:512]
                )
            for k in range(KT):
                nc.sync.dma_start(
                    xT_sb[:, k, 512:1024], xT_d[k * 128 : (k + 1) * 128, 512:1024]
                )
            for k in range(KT):
                nc.sync.dma_start(
                    xT_sb[:, k, 1024:2048], xT_d[k * 128 : (k + 1) * 128, 1024:2048]
                )
            for k in range(2):
                nc.sync.dma_start(wo_sb[:, k, :], wo_d[k * 128 : (k + 1) * 128, :])

            # qT: [64h + d, r, n] = head 2r+h;  kT2: k duplicated on both halves
            qT_sb = qpool.tile([128, 2, N], BF16)
            kT2_sb = qpool.tile([128, N], BF16)
            vT_sb = qpool.tile([128, N], BF16)     # only partitions 64:128 used
            vnat_sb = qpool.tile([128, 16, 66], BF16)  # [:, t, 0:64]=v, [:, t, 64]=1
            nc.vector.memset(vnat_sb[:, :, 64:65], 1.0)

            # ---- projections (all chunks upfront); k-outer so each x k-tile
            # is consumed by all 3 m-blocks as soon as its DMA lands ----
            with (
                tc.tile_pool(name="proj_ps", bufs=2, space="PSUM") as proj_ps,
                tc.tile_pool(name="tr_ps", bufs=2, space="PSUM") as tr_ps,
            ):
                for c in range(NCH):
                    sl = slice(c * 512, (c + 1) * 512)
                    ps = [
                        proj_ps.tile([128, 512], F32, name=f"ps{m}", tag=f"ps{m}")
                        for m in range(3)
                    ]
                    for k in range(KT):
                        for m in range(3):
                            nc.tensor.matmul(
                                ps[m][:],
                                wqkv_sb[:, k, m * 128 : (m + 1) * 128],
                                xT_sb[:, k, sl],
                                start=(k == 0),
                                stop=(k == KT - 1),
                            )
                    nc.vector.tensor_copy(vT_sb[64:128, sl], ps[2][64:128, :])
                    for t in range(4 * c, 4 * c + 4):
                        tp = tr_ps.tile([128, 64], BF16)
                        nc.tensor.transpose(
                            tp[:],
                            vT_sb[64:128, t * 128 : (t + 1) * 128],
                            ident_sb[64:128, :],
                        )
                        nc.vector.tensor_copy(vnat_sb[:, t, 0:64], tp[:])
                    nc.vector.tensor_copy(kT2_sb[0:64, sl], ps[2][0:64, :])
                    nc.scalar.copy(kT2_sb[64:128, sl], ps[2][0:64, :])
                    nc.vector.tensor_copy(qT_sb[:, 0, sl], ps[0][:])
                    nc.vector.tensor_copy(qT_sb[:, 1, sl], ps[1][:])

            y_parts = [
                dpool.tile([512, E], BF16, name=f"y_part{c}", tag=f"y_part{c}")
                for c in range(NCH)
            ]
            y_rss = dpool.tile([NCH, 128, E], BF16, name="y_rss", tag="y_rss")

            attnouts = {}
            pending_wo = []   # deque of (c, t, e) closures state
            wo_done = [0] * NCH

            # PSUM banks: qk 3x2 + pv 2x1 = 8.  Wo projection tiles borrow qk
            # pool slot halves; normalization and Wo work is queued and
            # drip-fed into the next round's instruction stream.
            with (
                tc.tile_pool(name="qk_ps", bufs=3, space="PSUM") as qk_ps,
                tc.tile_pool(name="pv_ps", bufs=2, space="PSUM") as pv_ps,
            ):
                pending = []

                def drain_one():
                    if pending:
                        pending.pop(0)()

                def emit_wo(c, t, e, yp, evict_act=False):
                    at = attnouts[c]
                    for kb in range(2):
                        nc.tensor.matmul(
                            yp,
                            at[:, kb, t * 128 : (t + 1) * 128],
                            wo_sb[:, kb, e * 512 : (e + 1) * 512],
                            start=(kb == 0),
                            stop=(kb == 1),
                        )
                    y_sb = ypool.tile([128, 512], BF16, name="y_sb", tag="y_sb")
                    if evict_act:
                        nc.scalar.copy(y_sb[:], yp)
                    else:
                        nc.vector.tensor_copy(y_sb[:], yp)
                    nc.sync.dma_start(
                        y_parts[c][t * 128 : (t + 1) * 128, e * 512 : (e + 1) * 512],
                        y_sb[:],
                    )
                    wo_done[c] += 1
                    if wo_done[c] == 8:
                        nc.gpsimd.collective_compute(
                            "ReduceScatter",
                            mybir.AluOpType.add,
                            replica_groups=GROUPS,
                            ins=[y_parts[c].opt()],
                            outs=[y_rss[c, :, :].opt()],
                        )

                def queue_wo(c, tiles, alt_evict=False):
                    # one closure per pair of (t, e) tiles sharing a qk slot
                    def go(pair=tuple(tiles), ea=alt_evict):
                        yp = qk_ps.tile([128, 2, 512], F32, name="qk", tag="qk")
                        for s, (t, e) in enumerate(pair):
                            emit_wo(c, t, e, yp[:, s, :], evict_act=(ea and s == 0))
                    pending.append(go)

                def queue_norm(c, r, h, pvc, at):
                    # broadcast 1/denominator across partitions with a tiny
                    # ones-matmul (PE has slack; keeps Pool out of the chain)
                    def go():
                        recip = rpool.tile([1, 512], F32R, name="recip", tag="recip")
                        with nc.allow_low_precision(reason="recip feeds f32r bcast mm"):
                            nc.vector.reciprocal(recip[:], pvc[64:65, :])
                        bcq = qk_ps.tile([128, 2, 512], F32, name="qk", tag="qk")
                        nc.tensor.matmul(
                            bcq[0:64, 0, :],
                            ones_sb[:],
                            recip[:],
                            start=True,
                            stop=True,
                        )
                        nc.vector.scalar_tensor_tensor(
                            out=at[64 * h : 64 * h + 64, r, :],
                            in0=pvc[0:64, :],
                            scalar=1.0,
                            in1=bcq[0:64, 0, :],
                            op0=MULT,
                            op1=MULT,
                        )
                    pending.append(go)

                for c in range(NCH):
                    nst = 4 * c + 4
                    at = apool.tile([128, 2, 512], F32R, name=f"attnoutT{c}", tag="at")
                    attnouts[c] = at
                    for r in range(2):
                        pv = [None, None]

                        def emit_pv(entry, pv=pv):
                            pi, pf0, ppT = entry
                            for h in range(2):
                                if pv[h] is None:
                                    pv[h] = pv_ps.tile(
                                        [65, 512], F32, name=f"pv{h}", tag="pv"
                                    )
                                nc.tensor.matmul(
                                    pv[h][:, pf0:512],
                                    vnat_sb[:, pi, 0:65],
                                    ppT[:, h, pf0:512],
                                    start=(pi == 0),
                                    stop=(pi == nst - 1),
                                )

                        pend = []
                        for i in range(nst):
                            f0 = max(0, 128 * (i - 4 * c))
                            diag = i >= 4 * c
                            qk = qk_ps.tile([128, 2, 512], F32, name="qk", tag="qk")
                            for h in range(2):
                                base = 64 * h
                                nc.tensor.matmul(
                                    qk[:, h, f0:512],
                                    kT2_sb[base : base + 64, i * 128 : (i + 1) * 128],
                                    qT_sb[base : base + 64, r,
                                          c * 512 + f0 : (c + 1) * 512],
                                    start=True,
                                    stop=True,
                                    skip_group_check=diag,
                                )
                                if diag:
                                    # causal mask applied on the PE: accumulate
                                    # maskT.T @ I = -1e30 upper triangle onto
                                    # the diagonal 128x128 score block — keeps
                                    # the exp dependency chain PE-only.
                                    nc.tensor.matmul(
                                        qk[:, h, f0 : f0 + 128],
                                        maskT_sb[:],
                                        identN_sb[:],
                                        start=False,
                                        stop=True,
                                        skip_group_check=True,
                                    )
                            if len(pend) == 2:
                                emit_pv(pend.pop(0))
                            pT = ppool.tile([128, 2, 512], BF16, name="pT", tag="pT")
                            nc.scalar.activation(
                                pT[:, :, f0:512], qk[:, :, f0:512], EXP
                            )
                            pend.append((i, f0, pT))
                            if i % 2 == 1:
                                drain_one()
                        while pend:
                            emit_pv(pend.pop(0))
                        for h in range(2):
                            # evict pv to SBUF immediately: frees the PSUM
                            # bank for the next round without waiting on the
                            # recip/bcast/normalize chain; the two heads run
                            # on Act/DVE in parallel (Act idles at boundaries)
                            pvc = vpool.tile([65, 512], F32, name="pvc", tag="pvc")
                            if h == 0:
                                nc.scalar.copy(pvc[:], pv[h][:])
                            else:
                                nc.vector.tensor_copy(pvc[:], pv[h][:])
                            queue_norm(c, r, h, pvc, at)
                    for t in range(4):
                        queue_wo(c, [(t, 0), (t, 1)], alt_evict=(c == NCH - 1))
                # tail: drain everything left (chunk 3 normalize + Wo)
                rest = pending[:]
                pending.clear()
                for go in rest:
                    go()
                # deferred output DMAs: chunks 0-2 together (deps force them
                # after chunk-2's stores), chunk 3 alone so the post-RS tail
                # only pays for one 256KB transfer.
                nc.sync.dma_start(y_rs_d[0:3, :, :], y_rss[0:3, :, :])
                nc.sync.dma_start(y_rs_d[3, :, :], y_rss[3, :, :])

    nc.finalize()
    return nc


def get_program():
    if "nc" not in _NC_CACHE:
        _NC_CACHE["nc"] = build_program()
    return _NC_CACHE["nc"]


def make_in_maps(x, Wq, Wk, Wv, Wo):
    bf16 = ml_dtypes.bfloat16
    tri = np.where(
        np.arange(128)[:, None] <= np.arange(128)[None, :], 0.0, -1e30
    ).astype(np.float32)
    maskT = np.ascontiguousarray(tri.T).astype(bf16)
    identN = np.eye(128, dtype=np.float32).astype(bf16)
    ident = np.tile(np.eye(64, dtype=np.float32), (2, 1)).astype(bf16)
    xT = [np.ascontiguousarray(x[b].T).astype(bf16) for b in range(B)]
    in_maps = []
    for core in range(8):
        b, j = core // 4, core % 4
        wqkv = np.ascontiguousarray(
            np.concatenate(
                [
                    Wq[:, j * 256 : (j + 1) * 256] * (1.0 / np.sqrt(D)),
                    Wk[:, j * 64 : (j + 1) * 64],
                    Wv[:, j * 64 : (j + 1) * 64],
                ],
                axis=1,
            )
        ).astype(bf16)
        wo = np.ascontiguousarray(Wo[j * 256 : (j + 1) * 256, :]).astype(np.float32)
        in_maps.append(
            {"xT": xT[b], "wqkv": wqkv, "wo": wo, "maskT": maskT,
             "identN": identN, "ident": ident}
        )
    return in_maps


def gather_output(results):
    y = np.empty((B, N, E), dtype=np.float32)
    for core in range(8):
        b, j = core // 4, core % 4
        piece = np.asarray(results[core]["y_rs"]).astype(np.float32)
        for c in range(NCH):
            r0 = 512 * c + 128 * j
            y[b, r0 : r0 + 128, :] = piece[c]
    return y


def kernel(x, Wq, Wk, Wv, Wo, _trace=False, **trace_kwargs):
    nc = get_program()
    in_maps = make_in_maps(
        np.asarray(x), np.asarray(Wq), np.asarray(Wk), np.asarray(Wv), np.asarray(Wo)
    )
    res = run_bass_kernel_spmd(nc, in_maps, list(range(8)), trace=_trace, **trace_kwargs)
    out = gather_output(res.results)
    if _trace:
        return out, res
    return out
